# revision 5
# baseline (speedup 1.0000x reference)
"""Trainium2 Bass kernel for ClosebyValuationFunction.

reference semantics (per row r of two [B, 6] f32 tensors):
    dis_x = |z1[r,4] - z2[r,4]|; dis_y = |z1[r,5] - z2[r,5]|
    out[r] = 0.99 if (dis_x < 2.0) & (dis_y <= 0.1) else 0.01

Strategy: data-parallel over 8 cores (B/8 rows each). Only columns 4/5
of each input participate, so the shard each core receives is the
projection of its row range onto those columns, interleaved into a
single [N, 4] tensor (x1, y1, x2, y2 per row) — the host does the
slice/pack while sharding; every arithmetic op (subtract, abs,
compare, select) runs on device. Per core that is 16 MiB in + 4 MiB
out of HBM traffic instead of the 52 MiB of full rows. The single
packed tensor means one input DMA per chunk with 16 KiB contiguous
per-partition lines, which keeps all 16 DMA engines near their
per-descriptor throughput limit.

Input DMAs ride the Sync HWDGE queue; output DMAs ride the ACT HWDGE
queue so a compute-gated store never stalls the input stream (HWDGE is
FIFO per issuing engine). |d| runs on ACT, the rest on DVE. The last
chunk is tapered into small sub-chunks to shrink the kernel tail.
"""

import numpy as np

B = 8388608
D = 4            # packed per-row shard: (x1, y1, x2, y2)
M = 8            # cores
N = B // M       # rows per core
P = 128          # partitions
E = 1024         # rows per partition per full chunk
E_TAIL = 256     # rows per partition per tail sub-chunk

HI = 0.99
LO = 0.01
X_THRESH = 2.0
Y_THRESH = 0.1

_cache: dict = {}


def _build(n_rows: int = N, e: int = E, e_tail: int = E_TAIL,
           io_bufs: int = 3, tail_bufs: int = 4, tmp_bufs: int = 3):
    """tail_bufs > 0 gives the small tail pieces their own tile pool with
    that many buffers (they otherwise share the big chunks' slots)."""
    from concourse import bacc, mybir
    from concourse.tile import TileContext

    f32 = mybir.dt.float32
    Alu = mybir.AluOpType
    Act = mybir.ActivationFunctionType

    n_chunks = n_rows // (P * e)
    assert n_chunks * P * e == n_rows
    assert e % e_tail == 0

    nc = bacc.Bacc("TRN2", target_bir_lowering=False, debug=False)

    xy = nc.dram_tensor("xy", [n_rows, D], f32, kind="ExternalInput")
    out = nc.dram_tensor("out", [n_rows], f32, kind="ExternalOutput")

    # full chunks: chunk c, partition p holds rows [(c*P + p)*e, ...)
    xyt = xy[:].rearrange("(c p e) d -> c p (e d)", p=P, e=e)
    outt = out[:].rearrange("(c p e) -> c p e", p=P, e=e)

    # geometric taper of the last chunk: shrinks the end-of-kernel
    # compute-chain drain that no remaining DMA can hide
    tail_sizes = []
    left = e
    while left > 2 * e_tail:
        tail_sizes.append(e_tail)
        left -= e_tail
    while left > 2 * (e_tail // 4):
        tail_sizes.append(e_tail // 2)
        left -= e_tail // 2
    tail_sizes += [e_tail // 4] * (left // (e_tail // 4))
    assert sum(tail_sizes) == e, (tail_sizes, e)
    tail_aps = []
    row0 = (n_chunks - 1) * P * e
    for sz in tail_sizes:
        zz = xy[row0:row0 + P * sz, :].rearrange(
            "(p e) d -> p (e d)", p=P, e=sz)
        oo = out[row0:row0 + P * sz].rearrange("(p e) -> p e", p=P, e=sz)
        tail_aps.append((zz, oo, sz))
        row0 += P * sz

    # squared thresholds for the all-DVE tail path; d*d <cmp> t*t is
    # bit-equivalent to |d| <cmp> t for these f32 thresholds (verified
    # exhaustively over the boundary neighborhoods)
    x_t2 = float(np.float32(X_THRESH) * np.float32(X_THRESH))
    y_t2 = float(np.float32(Y_THRESH) * np.float32(Y_THRESH))

    def piece(tc, io, tp, in_ap, out_ap, ecur, tag="", use_act=True):
        t = io.tile([P, D * ecur], f32, tag="xy" + tag)
        nc.sync.dma_start(out=t[:], in_=in_ap)

        v = t[:].rearrange("p (e d) -> p e d", d=D)

        dx = tp.tile([P, ecur], f32, tag="dx")
        dy = tp.tile([P, ecur], f32, tag="dy")
        nc.vector.tensor_tensor(
            out=dx[:], in0=v[:, :, 0], in1=v[:, :, 2], op=Alu.subtract
        )
        nc.vector.tensor_tensor(
            out=dy[:], in0=v[:, :, 1], in1=v[:, :, 3], op=Alu.subtract
        )
        if use_act:
            # |d| on ACT (overlaps with DVE), compare in place -> 1.0/0.0
            nc.scalar.activation(out=dx[:], in_=dx[:], func=Act.Abs)
            nc.scalar.activation(out=dy[:], in_=dy[:], func=Act.Abs)
            nc.vector.tensor_scalar(
                out=dx[:], in0=dx[:], scalar1=X_THRESH, scalar2=None,
                op0=Alu.is_lt,
            )
            nc.vector.tensor_scalar(
                out=dy[:], in0=dy[:], scalar1=Y_THRESH, scalar2=None,
                op0=Alu.is_le,
            )
        else:
            # all-DVE: square then compare vs squared threshold — avoids
            # two cross-engine round-trips on the end-of-kernel chain
            nc.vector.tensor_tensor(out=dx[:], in0=dx[:], in1=dx[:],
                                    op=Alu.mult)
            nc.vector.tensor_tensor(out=dy[:], in0=dy[:], in1=dy[:],
                                    op=Alu.mult)
            nc.vector.tensor_scalar(
                out=dx[:], in0=dx[:], scalar1=x_t2, scalar2=None,
                op0=Alu.is_lt,
            )
            nc.vector.tensor_scalar(
                out=dy[:], in0=dy[:], scalar1=y_t2, scalar2=None,
                op0=Alu.is_le,
            )
        # and
        nc.vector.tensor_tensor(out=dy[:], in0=dx[:], in1=dy[:], op=Alu.mult)
        # exact 0.99f/0.01f: max(w*0.99, 0.01)
        res = tp.tile([P, ecur], f32, tag="res")
        nc.vector.tensor_scalar(
            out=res[:], in0=dy[:], scalar1=HI, scalar2=LO,
            op0=Alu.mult, op1=Alu.max,
        )
        # store on the ACT HWDGE queue: doesn't block the input stream
        nc.scalar.dma_start(out=out_ap, in_=res[:])

    with TileContext(nc) as tc:
        from contextlib import ExitStack
        with ExitStack() as ctx:
            io = ctx.enter_context(tc.tile_pool(name="io", bufs=io_bufs))
            tp = ctx.enter_context(tc.tile_pool(name="tmp", bufs=tmp_bufs))
            tio = (
                ctx.enter_context(tc.tile_pool(name="tio", bufs=tail_bufs))
                if tail_bufs else io
            )
            for c in range(n_chunks - 1):
                piece(tc, io, tp, xyt[c], outt[c], e)
            for zz, oo, sz in tail_aps:
                piece(tc, tio, tp, zz, oo, sz,
                      tag="t" if tail_bufs else "", use_act=False)

    nc.finalize()
    return nc


def _pack(z_1: np.ndarray, z_2: np.ndarray) -> np.ndarray:
    """Shard prep: project both inputs onto columns 4/5 and interleave
    into one [B, 4] row-major array: (x1, y1, x2, y2) per row."""
    xy = np.empty((z_1.shape[0], 4), dtype=np.float32)
    xy[:, 0:2] = z_1[:, 4:6]
    xy[:, 2:4] = z_2[:, 4:6]
    return xy


def _run(z_1: np.ndarray, z_2: np.ndarray, trace: bool = False):
    from concourse.bass_utils import run_bass_kernel_spmd

    if "nc" not in _cache:
        _cache["nc"] = _build()
    nc = _cache["nc"]

    xy = _pack(np.asarray(z_1, dtype=np.float32),
               np.asarray(z_2, dtype=np.float32))
    in_maps = [{"xy": xy[i * N:(i + 1) * N]} for i in range(M)]
    r = run_bass_kernel_spmd(nc, in_maps, list(range(M)), trace=trace)
    out = np.concatenate([r.results[i]["out"] for i in range(M)], axis=0)
    return out, r


def kernel(z_1: np.ndarray, z_2: np.ndarray) -> np.ndarray:
    out, _ = _run(z_1, z_2, trace=False)
    return out


# revision 17
# speedup vs baseline: 1.1813x; 1.1813x over previous
"""Trainium2 Bass kernel for ClosebyValuationFunction.

reference semantics (per row r of two [B, 6] f32 tensors):
    dis_x = |z1[r,4] - z2[r,4]|; dis_y = |z1[r,5] - z2[r,5]|
    out[r] = 0.99 if (dis_x < 2.0) & (dis_y <= 0.1) else 0.01

Strategy: data-parallel over 8 cores (B/8 rows each). Only columns 4/5
of each input participate, so the shard each core receives is the
projection of its row range onto those columns, packed planar-pairs as
[2, N, 2] (plane 0 = z1's (x,y) pairs, plane 1 = z2's) — the host does
the slice/pack while sharding; every arithmetic op (subtract, abs,
compare, select) runs on device. Per core that is 16 MiB in + 4 MiB
out of HBM traffic instead of the 52 MiB of full rows.

Per chunk the compute is spread so no engine exceeds the DMA time:
one fused DVE subtract over both planes, |.| on ACT in one op, then
two DVE ops — cx = (|dx| < 2)*0.98 (tensor_scalar) and
res0 = (|dy| <= 0.1)*cx (scalar_tensor_tensor) — and the final
res = res0 + 0.01 on ACT (Identity+bias), which also issues the
store. DVE ~4.4us/chunk, ACT ~3.6us/chunk, DMA ~6.4us/chunk.

Input DMAs ride the Sync HWDGE queue; output DMAs ride the ACT HWDGE
queue so a compute-gated store never stalls the input stream (HWDGE is
FIFO per issuing engine). The last chunk is tapered into small
sub-chunks to shrink the kernel tail.
"""

import numpy as np

B = 8388608
M = 8            # cores
N = B // M       # rows per core
P = 128          # partitions
E = 1024         # rows per partition per full chunk
E_TAIL = 256     # rows per partition per tail sub-chunk

HI = 0.99
LO = 0.01
X_THRESH = 2.0
Y_NEXT = float(np.nextafter(np.float32(0.1), np.float32(1)))  # |dy|<=0.1 == |dy|<Y_NEXT

_cache: dict = {}


def _build(n_rows: int = N, e: int = E, e_tail: int = E_TAIL,
           io_bufs: int = 3, tail_bufs: int = 4, tmp_bufs: int = 3):
    from concourse import bacc, mybir
    from concourse.tile import TileContext

    f32 = mybir.dt.float32
    Alu = mybir.AluOpType
    Act = mybir.ActivationFunctionType

    n_chunks = n_rows // (P * e)
    assert n_chunks * P * e == n_rows
    assert e % e_tail == 0

    nc = bacc.Bacc("TRN2", target_bir_lowering=False, debug=False)

    xy = nc.dram_tensor("xy", [2, n_rows, 2], f32, kind="ExternalInput")
    out = nc.dram_tensor("out", [n_rows], f32, kind="ExternalOutput")

    # full chunks: chunk c, partition p holds rows [(c*P + p)*e, ...) of
    # both planes; SBUF free layout = [plane0 pairs (2e)][plane1 pairs (2e)]
    z1c = xy[0].rearrange("(c p e) d -> c p (e d)", p=P, e=e)
    z2c = xy[1].rearrange("(c p e) d -> c p (e d)", p=P, e=e)
    outt = out[:].rearrange("(c p e) -> c p e", p=P, e=e)

    # geometric taper of the last chunk: shrinks the end-of-kernel
    # compute-chain drain that no remaining DMA can hide
    tail_sizes = []
    left = e
    while left > 2 * e_tail:
        tail_sizes.append(e_tail)
        left -= e_tail
    while left > 2 * (e_tail // 4):
        tail_sizes.append(e_tail // 2)
        left -= e_tail // 2
    tail_sizes += [e_tail // 4] * (left // (e_tail // 4))
    assert sum(tail_sizes) == e, (tail_sizes, e)
    tail_aps = []
    row0 = (n_chunks - 1) * P * e
    for sz in tail_sizes:
        zz1 = xy[0, row0:row0 + P * sz, :].rearrange(
            "(p e) d -> p (e d)", p=P, e=sz)
        zz2 = xy[1, row0:row0 + P * sz, :].rearrange(
            "(p e) d -> p (e d)", p=P, e=sz)
        oo = out[row0:row0 + P * sz].rearrange("(p e) -> p e", p=P, e=sz)
        tail_aps.append((zz1, zz2, oo, sz))
        row0 += P * sz

    def piece(tc, io, tp, lo_ap, in1_ap, in2_ap, out_ap, ecur, tag=""):
        t = io.tile([P, 4 * ecur], f32, tag="xy" + tag)
        nc.sync.dma_start(out=t[:, 0:2 * ecur], in_=in1_ap)
        nc.sync.dma_start(out=t[:, 2 * ecur:4 * ecur], in_=in2_ap)

        v = t[:].rearrange("p (s e d) -> p s e d", s=2, d=2)

        d_ = tp.tile([P, 2 * ecur], f32, tag="d")
        # one DVE subtract over both planes; (dx, dy) stay interleaved
        nc.vector.tensor_tensor(
            out=d_[:], in0=v[:, 0, :, :], in1=v[:, 1, :, :],
            op=Alu.subtract)
        dv = d_[:].rearrange("p (e d) -> p e d", d=2)
        ax = dv[:, :, 0]
        ay = dv[:, :, 1]
        nc.scalar.activation(out=d_[:], in_=d_[:], func=Act.Abs)

        # cx = (|dx| < 2) * (HI - LO)  ->  {0.98, 0}
        cx = tp.tile([P, ecur], f32, tag="cx")
        nc.vector.tensor_scalar(
            out=cx[:], in0=ax, scalar1=X_THRESH, scalar2=HI - LO,
            op0=Alu.is_lt, op1=Alu.mult)
        # res0 = (|dy| <= 0.1) * cx  (one fused DVE op)
        res0 = tp.tile([P, ecur], f32, tag="res0")
        nc.vector.scalar_tensor_tensor(
            out=res0[:], in0=ay, scalar=float(np.float32(0.1)),
            in1=cx[:], op0=Alu.is_le, op1=Alu.mult)
        # res = res0 + LO on ACT: keeps the final select off DVE, and the
        # store is issued by the same engine right after
        res = tp.tile([P, ecur], f32, tag="res")
        nc.scalar.activation(out=res[:], in_=res0[:], func=Act.Identity,
                             bias=lo_ap)
        # store on the ACT HWDGE queue: doesn't block the input stream
        nc.scalar.dma_start(out=out_ap, in_=res[:])

    with TileContext(nc) as tc:
        from contextlib import ExitStack
        with ExitStack() as ctx:
            cp = ctx.enter_context(tc.tile_pool(name="const", bufs=1))
            lo_t = cp.tile([P, 1], f32, tag="lo")
            nc.gpsimd.memset(lo_t[:], LO)
            io = ctx.enter_context(tc.tile_pool(name="io", bufs=io_bufs))
            tp = ctx.enter_context(tc.tile_pool(name="tmp", bufs=tmp_bufs))
            tio = (
                ctx.enter_context(tc.tile_pool(name="tio", bufs=tail_bufs))
                if tail_bufs else io
            )
            for c in range(n_chunks - 1):
                piece(tc, io, tp, lo_t[:], z1c[c], z2c[c], outt[c], e)
            for zz1, zz2, oo, sz in tail_aps:
                piece(tc, tio, tp, lo_t[:], zz1, zz2, oo, sz,
                      tag="t" if tail_bufs else "")

    nc.finalize()
    return nc


def _pack(z_1: np.ndarray, z_2: np.ndarray) -> np.ndarray:
    """Shard prep: per core, planes [2, N, 2] = (z1 xy pairs, z2 xy pairs)."""
    arr = np.empty((M, 2, N, 2), dtype=np.float32)
    for i in range(M):
        arr[i, 0] = z_1[i * N:(i + 1) * N, 4:6]
        arr[i, 1] = z_2[i * N:(i + 1) * N, 4:6]
    return arr


def _run(z_1: np.ndarray, z_2: np.ndarray, trace: bool = False, **bkw):
    from concourse.bass_utils import run_bass_kernel_spmd

    key = tuple(sorted(bkw.items()))
    if key not in _cache:
        _cache[key] = _build(**bkw)
    nc = _cache[key]

    arr = _pack(np.asarray(z_1, dtype=np.float32),
                np.asarray(z_2, dtype=np.float32))
    in_maps = [{"xy": arr[i]} for i in range(M)]
    r = run_bass_kernel_spmd(nc, in_maps, list(range(M)), trace=trace)
    out = np.concatenate([r.results[i]["out"] for i in range(M)], axis=0)
    return out, r


def kernel(z_1: np.ndarray, z_2: np.ndarray) -> np.ndarray:
    out, _ = _run(z_1, z_2, trace=False)
    return out
